# revision 2
# baseline (speedup 1.0000x reference)
"""MDN head (dense_mlp) Trainium2 Bass kernel.

reference:
    z_h   = tanh(x @ W1 + b1)                  [B, H]
    pi    = softmax(z_h @ Wpi + bpi)           [B, K]
    sigma = diag_embed(exp(z_h @ Wsig + bsig)) [B, K, D, D]
    mu    = (z_h @ Wmu + bmu)                  [B, K, D]

B=65536, COND=32, D=16, H=256, K=16. Pure data parallel over 8 cores
(8192 rows each). The kernel is memory-bound on the 1 GiB sigma output;
sigma rows are built in persistent SBUF tiles that are zeroed once, with
only the 256 diagonal positions (stride 17 within each 16x16 block)
rewritten per 128-row tile, then shipped with contiguous 2 MiB DMAs.
"""

import numpy as np

B, COND, D, H, K = 65536, 32, 16, 256, 16
N_CORES = 8
BC = B // N_CORES          # 8192 rows per core
P = 128                    # partition tile
NT = BC // P               # 64 tiles per core
KD = K * D                 # 256
KDD = K * D * D            # 4096

_NC_CACHE = {}


def _legalize_waits(nc, max_waits=1):
    """This container's walrus only supports one sync-wait per instruction
    (setupSyncWait TPB_CTRL_NO limit). Hoist excess waits onto same-engine
    no-op carriers inserted immediately before the owning instruction."""
    import concourse.mybir as mybir

    n_split = 0
    fn = nc.m.functions[0]
    for blk in fn.blocks:
        new_insts = []
        changed = False
        for inst in blk.instructions:
            si = inst.sync_info
            waits = list(si.on_wait) if si and si.on_wait else []
            if len(waits) > max_waits:
                extra, keep = waits[:-max_waits], waits[-max_waits:]
                for j, w in enumerate(extra):
                    carrier = mybir.InstNoOp(
                        name=f"{inst.name}-wsplit{j}",
                        ins=[],
                        outs=[],
                        sync_info=mybir.SyncInfo(on_wait=[w], on_update=[]),
                    )
                    carrier.engine = inst.engine
                    new_insts.append(carrier)
                    n_split += 1
                si.on_wait = keep
                changed = True
            new_insts.append(inst)
        if changed:
            blk.instructions[:] = new_insts
    return n_split


def _build_nc(sig_bufs=2, tmp_bufs=3):
    import concourse.bass as bass
    import concourse.tile as tile
    from concourse import mybir
    from concourse.masks import make_identity

    f32 = mybir.dt.float32
    AF = mybir.ActivationFunctionType

    nc = bass.Bass()
    x = nc.dram_tensor("x", [BC, COND], f32, kind="ExternalInput")
    W1 = nc.dram_tensor("W1", [COND, H], f32, kind="ExternalInput")
    b1 = nc.dram_tensor("b1", [H], f32, kind="ExternalInput")
    Wpi = nc.dram_tensor("Wpi", [H, K], f32, kind="ExternalInput")
    bpi = nc.dram_tensor("bpi", [K], f32, kind="ExternalInput")
    Wsig = nc.dram_tensor("Wsig", [H, KD], f32, kind="ExternalInput")
    bsig = nc.dram_tensor("bsig", [KD], f32, kind="ExternalInput")
    Wmu = nc.dram_tensor("Wmu", [H, KD], f32, kind="ExternalInput")
    bmu = nc.dram_tensor("bmu", [KD], f32, kind="ExternalInput")
    pi_o = nc.dram_tensor("pi", [BC, K], f32, kind="ExternalOutput")
    sig_o = nc.dram_tensor("sigma", [BC, KDD], f32, kind="ExternalOutput")
    mu_o = nc.dram_tensor("mu", [BC, KD], f32, kind="ExternalOutput")

    with tile.TileContext(nc) as tc:
        with (
            tc.tile_pool(name="const", bufs=1) as const,
            tc.tile_pool(name="sigbg", bufs=1) as sigbg,
            tc.tile_pool(name="xin", bufs=tmp_bufs) as xin,
            tc.tile_pool(name="xt", bufs=tmp_bufs) as xtp,
            tc.tile_pool(name="zh", bufs=tmp_bufs) as zhp,
            tc.tile_pool(name="sout", bufs=tmp_bufs) as soutp,
            tc.tile_pool(name="muout", bufs=tmp_bufs) as muoutp,
            tc.tile_pool(name="piout", bufs=4) as pioutp,
            tc.tile_pool(name="ps_xt", bufs=1, space="PSUM") as ps_xt,
            tc.tile_pool(name="ps_z", bufs=2, space="PSUM") as ps_z,
            tc.tile_pool(name="ps_pi", bufs=1, space="PSUM") as ps_pi,
            tc.tile_pool(name="ps_sig", bufs=2, space="PSUM") as ps_sig,
            tc.tile_pool(name="ps_mu", bufs=2, space="PSUM") as ps_mu,
        ):
            # ---- one-time constants ----
            ident = const.tile([P, P], f32)
            make_identity(nc, ident)

            w1_sb = const.tile([COND, H], f32)
            nc.gpsimd.dma_start(out=w1_sb, in_=W1[:, :])
            # b1 per-partition chunks: [128, 2], chunk c holds b1[c*128+p]
            b1_sb = const.tile([P, 2], f32)
            nc.gpsimd.dma_start(out=b1_sb, in_=b1[:].rearrange("(c p) -> p c", p=P))
            # layer-2 weights, H-chunk-major: [128, 2, N]
            wpi_sb = const.tile([P, 2, K], f32)
            nc.gpsimd.dma_start(
                out=wpi_sb, in_=Wpi[:, :].rearrange("(c p) k -> p c k", p=P)
            )
            wsig_sb = const.tile([P, 2, KD], f32)
            nc.gpsimd.dma_start(
                out=wsig_sb, in_=Wsig[:, :].rearrange("(c p) n -> p c n", p=P)
            )
            wmu_sb = const.tile([P, 2, KD], f32)
            nc.gpsimd.dma_start(
                out=wmu_sb, in_=Wmu[:, :].rearrange("(c p) n -> p c n", p=P)
            )
            # bias rows for the K=1 ones-trick matmul
            bpi_sb = const.tile([1, K], f32)
            nc.gpsimd.dma_start(out=bpi_sb, in_=bpi[:].unsqueeze(0))
            bsig_sb = const.tile([1, KD], f32)
            nc.gpsimd.dma_start(out=bsig_sb, in_=bsig[:].unsqueeze(0))
            bmu_sb = const.tile([1, KD], f32)
            nc.gpsimd.dma_start(out=bmu_sb, in_=bmu[:].unsqueeze(0))
            ones_sb = const.tile([1, P], f32)
            nc.vector.memset(ones_sb, 1.0)

            # persistent sigma background tiles: zeroed once, only diagonal
            # positions rewritten each iteration
            sig_tiles = []
            for i in range(sig_bufs):
                t = sigbg.tile([P, KDD], f32, tag=f"sigbg{i}")
                nc.vector.memset(t, 0.0)
                sig_tiles.append(t)

            # ---- main loop over 64 row-tiles ----
            for t in range(NT):
                rows = slice(t * P, (t + 1) * P)

                # load x tile (natural layout) and transpose on PE
                x_nat = xin.tile([P, COND], f32)
                nc.scalar.dma_start(out=x_nat, in_=x[rows, :])
                xt_ps = ps_xt.tile([COND, P], f32)
                nc.tensor.transpose(xt_ps, x_nat, ident)
                xt_sb = xtp.tile([COND, P], f32)
                nc.vector.tensor_copy(out=xt_sb, in_=xt_ps)

                # layer 1: z_hT chunks [h=128, b=128]; tanh with per-partition bias
                zh_sb = zhp.tile([P, 2, P], f32)
                for c in range(2):
                    z_ps = ps_z.tile([P, P], f32)
                    nc.tensor.matmul(
                        z_ps,
                        w1_sb[:, c * P : (c + 1) * P],
                        xt_sb,
                        start=True,
                        stop=True,
                    )
                    nc.scalar.activation(
                        out=zh_sb[:, c, :],
                        in_=z_ps,
                        func=AF.Tanh,
                        bias=b1_sb[:, c : c + 1],
                        scale=1.0,
                    )

                # layer 2: natural-layout heads via lhsT = z_hT chunks,
                # bias folded as a K=1 matmul with a ones row
                pi_ps = ps_pi.tile([P, K], f32)
                sig_ps = ps_sig.tile([P, KD], f32)
                mu_ps = ps_mu.tile([P, KD], f32)
                for ps, w_sb, bias_sb in (
                    (pi_ps, wpi_sb, bpi_sb),
                    (sig_ps, wsig_sb, bsig_sb),
                    (mu_ps, wmu_sb, bmu_sb),
                ):
                    nc.tensor.matmul(
                        ps, zh_sb[:, 0, :], w_sb[:, 0, :], start=True, stop=False
                    )
                    nc.tensor.matmul(
                        ps, zh_sb[:, 1, :], w_sb[:, 1, :], start=False, stop=False
                    )
                    nc.tensor.matmul(ps, ones_sb, bias_sb, start=False, stop=True)

                # sigma: exp, scatter diagonal into persistent tile, ship
                s_sb = soutp.tile([P, KD], f32)
                nc.scalar.activation(out=s_sb, in_=sig_ps, func=AF.Exp)
                bg = sig_tiles[t % sig_bufs]
                nc.vector.tensor_copy(
                    out=bg.rearrange("p (k e) -> p k e", k=K)[:, :, ::D + 1],
                    in_=s_sb.rearrange("p (k i) -> p k i", k=K),
                )
                dma_eng = nc.sync if t % 2 == 0 else nc.scalar
                dma_eng.dma_start(out=sig_o[rows, :], in_=bg)

                # mu: copy out of PSUM, ship
                mu_sb = muoutp.tile([P, KD], f32)
                nc.scalar.activation(out=mu_sb, in_=mu_ps, func=AF.Copy)
                nc.sync.dma_start(out=mu_o[rows, :], in_=mu_sb)

                # pi: softmax over free dim (K=16)
                pmax = pioutp.tile([P, 1], f32, tag="pmax")
                nc.vector.reduce_max(
                    out=pmax, in_=pi_ps, axis=mybir.AxisListType.X
                )
                pe_sb = pioutp.tile([P, K], f32, tag="pexp")
                nc.vector.tensor_scalar(
                    out=pe_sb,
                    in0=pi_ps,
                    scalar1=pmax,
                    scalar2=None,
                    op0=mybir.AluOpType.subtract,
                )
                nc.scalar.activation(out=pe_sb, in_=pe_sb, func=AF.Exp)
                psum_sb = pioutp.tile([P, 1], f32, tag="psum")
                nc.vector.reduce_sum(
                    out=psum_sb, in_=pe_sb, axis=mybir.AxisListType.X
                )
                pinv = pioutp.tile([P, 1], f32, tag="pinv")
                nc.vector.reciprocal(out=pinv, in_=psum_sb)
                pi_sb = pioutp.tile([P, K], f32, tag="pisb")
                nc.vector.tensor_scalar_mul(out=pi_sb, in0=pe_sb, scalar1=pinv)
                nc.gpsimd.dma_start(out=pi_o[rows, :], in_=pi_sb)

    _legalize_waits(nc)
    return nc


def kernel(x, W1, b1, Wpi, bpi, Wsig, bsig, Wmu, bmu):
    from concourse.bass_utils import run_bass_kernel_spmd

    key = "nc"
    if key not in _NC_CACHE:
        _NC_CACHE[key] = _build_nc()
    nc = _NC_CACHE[key]

    x = np.ascontiguousarray(np.asarray(x, dtype=np.float32))
    weights = {
        "W1": np.ascontiguousarray(np.asarray(W1, np.float32)),
        "b1": np.ascontiguousarray(np.asarray(b1, np.float32)),
        "Wpi": np.ascontiguousarray(np.asarray(Wpi, np.float32)),
        "bpi": np.ascontiguousarray(np.asarray(bpi, np.float32)),
        "Wsig": np.ascontiguousarray(np.asarray(Wsig, np.float32)),
        "bsig": np.ascontiguousarray(np.asarray(bsig, np.float32)),
        "Wmu": np.ascontiguousarray(np.asarray(Wmu, np.float32)),
        "bmu": np.ascontiguousarray(np.asarray(bmu, np.float32)),
    }
    in_maps = [
        {"x": x[c * BC : (c + 1) * BC], **weights} for c in range(N_CORES)
    ]
    res = run_bass_kernel_spmd(nc, in_maps, core_ids=list(range(N_CORES)))
    pi = np.concatenate([r["pi"] for r in res.results], axis=0)
    sigma = np.concatenate([r["sigma"] for r in res.results], axis=0)
    mu = np.concatenate([r["mu"] for r in res.results], axis=0)
    return (
        pi.reshape(B, K),
        sigma.reshape(B, K, D, D),
        mu.reshape(B, K, D),
    )


# revision 5
# speedup vs baseline: 1.4076x; 1.4076x over previous
"""MDN head (dense_mlp) Trainium2 Bass kernel.

reference:
    z_h   = tanh(x @ W1 + b1)                  [B, H]
    pi    = softmax(z_h @ Wpi + bpi)           [B, K]
    sigma = diag_embed(exp(z_h @ Wsig + bsig)) [B, K, D, D]
    mu    = (z_h @ Wmu + bmu)                  [B, K, D]

B=65536, COND=32, D=16, H=256, K=16. Pure data parallel over 8 cores
(8192 rows each). The kernel is memory-bound on the 1 GiB sigma output;
sigma rows are built in persistent SBUF tiles that are zeroed once, with
only the 256 diagonal positions (stride 17 within each 16x16 block)
rewritten per 128-row tile, then shipped with contiguous 2 MiB DMAs.
"""

import numpy as np

B, COND, D, H, K = 65536, 32, 16, 256, 16
N_CORES = 8
BC = B // N_CORES          # 8192 rows per core
P = 128                    # partition tile
NT = BC // P               # 64 tiles per core
KD = K * D                 # 256
KDD = K * D * D            # 4096

_NC_CACHE = {}


def _legalize_waits(nc, max_waits=1):
    """This container's walrus only supports one sync-wait per instruction
    (setupSyncWait TPB_CTRL_NO limit). Hoist excess waits onto same-engine
    no-op carriers inserted immediately before the owning instruction."""
    import concourse.mybir as mybir

    n_split = 0
    fn = nc.m.functions[0]
    for blk in fn.blocks:
        new_insts = []
        changed = False
        for inst in blk.instructions:
            si = inst.sync_info
            waits = list(si.on_wait) if si and si.on_wait else []
            if len(waits) > max_waits:
                extra, keep = waits[:-max_waits], waits[-max_waits:]
                for j, w in enumerate(extra):
                    carrier = mybir.InstNoOp(
                        name=f"{inst.name}-wsplit{j}",
                        ins=[],
                        outs=[],
                        sync_info=mybir.SyncInfo(on_wait=[w], on_update=[]),
                    )
                    carrier.engine = inst.engine
                    new_insts.append(carrier)
                    n_split += 1
                si.on_wait = keep
                changed = True
            new_insts.append(inst)
        if changed:
            blk.instructions[:] = new_insts
    return n_split


def _build_nc(sig_bufs=2, tmp_bufs=3):
    import concourse.bass as bass
    import concourse.tile as tile
    from concourse import mybir

    f32 = mybir.dt.float32
    bf16 = mybir.dt.bfloat16
    AF = mybir.ActivationFunctionType

    nc = bass.Bass()
    x = nc.dram_tensor("x", [BC, COND], f32, kind="ExternalInput")
    W1 = nc.dram_tensor("W1", [COND, H], f32, kind="ExternalInput")
    b1 = nc.dram_tensor("b1", [H], f32, kind="ExternalInput")
    Wpi = nc.dram_tensor("Wpi", [H, K], f32, kind="ExternalInput")
    bpi = nc.dram_tensor("bpi", [K], f32, kind="ExternalInput")
    Wsig = nc.dram_tensor("Wsig", [H, KD], f32, kind="ExternalInput")
    bsig = nc.dram_tensor("bsig", [KD], f32, kind="ExternalInput")
    Wmu = nc.dram_tensor("Wmu", [H, KD], f32, kind="ExternalInput")
    bmu = nc.dram_tensor("bmu", [KD], f32, kind="ExternalInput")
    pi_o = nc.dram_tensor("pi", [BC, K], f32, kind="ExternalOutput")
    sig_o = nc.dram_tensor("sigma", [BC, KDD], f32, kind="ExternalOutput")
    mu_o = nc.dram_tensor("mu", [BC, KD], f32, kind="ExternalOutput")

    PM = K + KD  # 272: pi || mu packed along the free dim

    with tile.TileContext(nc) as tc:
        with (
            tc.tile_pool(name="const", bufs=1) as const,
            tc.tile_pool(name="sigbg", bufs=1) as sigbg,
            tc.tile_pool(name="xin", bufs=tmp_bufs) as xin,
            tc.tile_pool(name="xt", bufs=tmp_bufs) as xtp,
            tc.tile_pool(name="zh", bufs=tmp_bufs) as zhp,
            tc.tile_pool(name="sout", bufs=tmp_bufs) as soutp,
            tc.tile_pool(name="muout", bufs=tmp_bufs) as muoutp,
            tc.tile_pool(name="piout", bufs=4) as pioutp,
            tc.tile_pool(name="ps_z", bufs=2, space="PSUM") as ps_z,
            tc.tile_pool(name="ps_pimu", bufs=3, space="PSUM") as ps_pimu,
            tc.tile_pool(name="ps_sig", bufs=3, space="PSUM") as ps_sig,
        ):
            # ---- one-time constants (weights cast to bf16 during DMA) ----
            w1_sb = const.tile([COND, H], bf16)
            nc.gpsimd.dma_start(out=w1_sb, in_=W1[:, :])
            # b1 per-partition chunks: [128, 2], chunk c holds b1[c*128+p]
            b1_sb = const.tile([P, 2], f32)
            nc.gpsimd.dma_start(out=b1_sb, in_=b1[:].rearrange("(c p) -> p c", p=P))
            # layer-2 weights, H-chunk-major; pi and mu packed side by side
            w2_sb = const.tile([P, 2, PM], bf16)
            nc.gpsimd.dma_start(
                out=w2_sb[:, :, 0:K],
                in_=Wpi[:, :].rearrange("(c p) k -> p c k", p=P),
            )
            nc.gpsimd.dma_start(
                out=w2_sb[:, :, K:PM],
                in_=Wmu[:, :].rearrange("(c p) n -> p c n", p=P),
            )
            wsig_sb = const.tile([P, 2, KD], bf16)
            nc.gpsimd.dma_start(
                out=wsig_sb, in_=Wsig[:, :].rearrange("(c p) n -> p c n", p=P)
            )
            # bias rows for the K=1 ones-trick matmul
            b2_sb = const.tile([1, PM], bf16)
            nc.gpsimd.dma_start(out=b2_sb[:, 0:K], in_=bpi[:].unsqueeze(0))
            nc.gpsimd.dma_start(out=b2_sb[:, K:PM], in_=bmu[:].unsqueeze(0))
            bsig_sb = const.tile([1, KD], bf16)
            nc.gpsimd.dma_start(out=bsig_sb, in_=bsig[:].unsqueeze(0))
            ones_sb = const.tile([1, P], bf16)
            nc.vector.memset(ones_sb, 1.0)

            # persistent sigma background tiles: zeroed once, only diagonal
            # positions rewritten each iteration
            sig_tiles = []
            for i in range(sig_bufs):
                t = sigbg.tile([P, KDD], f32, tag=f"sigbg{i}")
                nc.vector.memset(t, 0.0)
                sig_tiles.append(t)

            # ---- main loop over 64 row-tiles ----
            for t in range(NT):
                rows = slice(t * P, (t + 1) * P)

                # load x tile cast to bf16, transpose 32x32 blocks on DVE
                x_nat = xin.tile([P, COND], bf16)
                nc.gpsimd.dma_start(out=x_nat, in_=x[rows, :])
                xt_sb = xtp.tile([COND, P], bf16)
                for a in range(P // 32):
                    nc.vector.transpose(
                        out=xt_sb[0:32, 32 * a : 32 * (a + 1)],
                        in_=x_nat[32 * a : 32 * (a + 1), 0:32],
                    )

                # layer 1: z_hT chunks [h=128, b=128]; tanh with per-partition bias
                zh_sb = zhp.tile([P, 2, P], bf16)
                for c in range(2):
                    z_ps = ps_z.tile([P, P], f32)
                    nc.tensor.matmul(
                        z_ps,
                        w1_sb[:, c * P : (c + 1) * P],
                        xt_sb,
                        start=True,
                        stop=True,
                    )
                    nc.scalar.activation(
                        out=zh_sb[:, c, :],
                        in_=z_ps,
                        func=AF.Tanh,
                        bias=b1_sb[:, c : c + 1],
                        scale=1.0,
                    )

                # layer 2: natural-layout heads via lhsT = z_hT chunks,
                # bias folded as a K=1 matmul with a ones row
                pm_ps = ps_pimu.tile([P, PM], f32)
                sig_ps = ps_sig.tile([P, KD], f32)
                for ps, w_sb, bias_sb in (
                    (pm_ps, w2_sb, b2_sb),
                    (sig_ps, wsig_sb, bsig_sb),
                ):
                    nc.tensor.matmul(
                        ps, zh_sb[:, 0, :], w_sb[:, 0, :], start=True, stop=False
                    )
                    nc.tensor.matmul(
                        ps, zh_sb[:, 1, :], w_sb[:, 1, :], start=False, stop=False
                    )
                    nc.tensor.matmul(ps, ones_sb, bias_sb, start=False, stop=True)

                # sigma: exp, scatter diagonal into persistent tile, ship
                s_sb = soutp.tile([P, KD], f32)
                nc.scalar.activation(out=s_sb, in_=sig_ps, func=AF.Exp)
                bg = sig_tiles[t % sig_bufs]
                nc.vector.tensor_copy(
                    out=bg.rearrange("p (k e) -> p k e", k=K)[:, :, ::D + 1],
                    in_=s_sb.rearrange("p (k i) -> p k i", k=K),
                )
                dma_eng = nc.sync if t % 2 == 0 else nc.scalar
                dma_eng.dma_start(out=sig_o[rows, :], in_=bg)

                # mu: copy out of PSUM, ship
                mu_sb = muoutp.tile([P, KD], f32)
                nc.scalar.activation(out=mu_sb, in_=pm_ps[:, K:PM], func=AF.Copy)
                nc.sync.dma_start(out=mu_o[rows, :], in_=mu_sb)

                # pi: softmax over free dim (K=16)
                pmax = pioutp.tile([P, 1], f32, tag="pmax")
                nc.vector.reduce_max(
                    out=pmax, in_=pm_ps[:, 0:K], axis=mybir.AxisListType.X
                )
                pe_sb = pioutp.tile([P, K], f32, tag="pexp")
                nc.vector.tensor_scalar(
                    out=pe_sb,
                    in0=pm_ps[:, 0:K],
                    scalar1=pmax,
                    scalar2=None,
                    op0=mybir.AluOpType.subtract,
                )
                nc.scalar.activation(out=pe_sb, in_=pe_sb, func=AF.Exp)
                psum_sb = pioutp.tile([P, 1], f32, tag="psum")
                nc.vector.reduce_sum(
                    out=psum_sb, in_=pe_sb, axis=mybir.AxisListType.X
                )
                pinv = pioutp.tile([P, 1], f32, tag="pinv")
                nc.vector.reciprocal(out=pinv, in_=psum_sb)
                pi_sb = pioutp.tile([P, K], f32, tag="pisb")
                nc.vector.tensor_scalar_mul(out=pi_sb, in0=pe_sb, scalar1=pinv)
                nc.gpsimd.dma_start(out=pi_o[rows, :], in_=pi_sb)

    _legalize_waits(nc)
    return nc


def kernel(x, W1, b1, Wpi, bpi, Wsig, bsig, Wmu, bmu):
    from concourse.bass_utils import run_bass_kernel_spmd

    key = "nc"
    if key not in _NC_CACHE:
        _NC_CACHE[key] = _build_nc()
    nc = _NC_CACHE[key]

    x = np.ascontiguousarray(np.asarray(x, dtype=np.float32))
    weights = {
        "W1": np.ascontiguousarray(np.asarray(W1, np.float32)),
        "b1": np.ascontiguousarray(np.asarray(b1, np.float32)),
        "Wpi": np.ascontiguousarray(np.asarray(Wpi, np.float32)),
        "bpi": np.ascontiguousarray(np.asarray(bpi, np.float32)),
        "Wsig": np.ascontiguousarray(np.asarray(Wsig, np.float32)),
        "bsig": np.ascontiguousarray(np.asarray(bsig, np.float32)),
        "Wmu": np.ascontiguousarray(np.asarray(Wmu, np.float32)),
        "bmu": np.ascontiguousarray(np.asarray(bmu, np.float32)),
    }
    in_maps = [
        {"x": x[c * BC : (c + 1) * BC], **weights} for c in range(N_CORES)
    ]
    res = run_bass_kernel_spmd(nc, in_maps, core_ids=list(range(N_CORES)))
    pi = np.concatenate([r["pi"] for r in res.results], axis=0)
    sigma = np.concatenate([r["sigma"] for r in res.results], axis=0)
    mu = np.concatenate([r["mu"] for r in res.results], axis=0)
    return (
        pi.reshape(B, K),
        sigma.reshape(B, K, D, D),
        mu.reshape(B, K, D),
    )
